# revision 15
# baseline (speedup 1.0000x reference)
"""Bayesian linear layer on 8 TRN2 NeuronCores.

Computes  out = x @ (mu + softplus(rho) * eps_w).T + (bmu + softplus(brho) * eps_b)
for x [16384, 4096], weights [4096, 4096].

Sharding: 2-way split of the batch dim (N) x 4-way split of out_features.
Each core computes an [8192, 1024] fp32 output shard.

Design notes (v2):
  - Weight inputs are shipped host-transposed ([in_f, out_f] fp16), so the
    device materializes W^T = mu + softplus(rho)*eps with cheap LINEAR loads
    (0.6us SP dispatch per chunk vs 1.3us for a DMA transpose) and the
    elementwise softplus/FMA run directly in [i, o] layout.  Weights live in
    8 resident quad tiles [128, 4x1024] fp16; softplus is Exp then Ln(x+1)
    on ACT at FD=4096 to amortize the 352-cycle instruction overhead.
  - x is shipped host-transposed ([in_f, n] fp16) so k-major panels load
    with linear [128 x 1KB] DIRECT2D chunks (~0.6us SP dispatch each) into
    double-buffered k-quarter panels (NB=512 row super-tiles).  A DMA
    transpose dispatch costs a fixed ~1.3us of SP time regardless of rows,
    so 512 of them (measured 656us) would starve the SP ring.
  - Matmuls are fp16, N=512 moving, fp32 PSUM.  Phase = (super-tile, q-half);
    q=0 phases use PSUM banks 0-3, q=1 banks 4-7, and each bank is drained
    (DVE bias-add) right after its 32-matmul k-chain, so phase transitions
    never wait on banks.  Super-tile 0 instead interleaves both q halves
    across all 8 banks in k-arrival order, so the PE starts consuming weight
    quads ~10us in, overlapping the whole prep stream.
  - bias = bmu + softplus(brho)*eps_b is computed on one partition from
    [1, OS] rows and broadcast to [128, OS] with a K=1 ones-matmul.
All DMAs stay on the SP HWDGE ring: splitting across the SP+ACT rings
corrupts results on this stack (completion tracking assumes one ring).
"""

import numpy as np

import bass_rust as _bass_rust
import concourse.bacc as bacc
import concourse.tile as tile
from concourse import mybir
from concourse import bass_utils
from concourse.hw_specs import get_activation_tables


class _Bacc(bacc.Bacc):
    """Bacc whose activation-table placement resolves Exp and Ln to the one
    table set containing both (natural_log_exp_and_others), instead of
    thrashing between per-function sets (one 1.3us ACT_TABLE_LOAD per
    ACTIVATE)."""

    def insert_act_table_loads(self):
        tables = list(get_activation_tables(self.m.arch).items())
        AF = mybir.ActivationFunctionType
        filtered = []
        for name, funcs in tables:
            if name != "natural_log_exp_and_others":
                funcs = funcs - {AF.Exp, AF.Ln}
            filtered.append((name, funcs))
        _bass_rust.insert_act_table_loads(self, filtered)


R, C = 2, 4                      # grid: R-way split of N, C-way split of out_f
N, IN_F, OUT_F = 16384, 4096, 4096
NS, OS = N // R, OUT_F // C      # per-core shards: 8192 rows, 1024 out cols
KB = IN_F // 128                 # 32 k-blocks
NB = 512                         # rows per super-tile
SUBS = NB // 128                 # 4 row-subtiles per super-tile
NSUP = NS // NB                  # 16 super-tiles
NKQ = 4                          # k-quarters per super-tile panel set
KQ = KB // NKQ                   # 8 k-blocks per quarter
QUAD = 4                         # k-blocks per weight quad tile
NQUADS = KB // QUAD              # 8 weight quad tiles
N_CORES = 8

FP32 = mybir.dt.float32
F16 = mybir.dt.float16


def _build_nc():
    nc = _Bacc("TRN2", target_bir_lowering=False, debug=False)

    xT = nc.dram_tensor("xT", [IN_F, NS], F16, kind="ExternalInput").ap()
    # host-transposed weight params: [in_f, out_f] for this core's o-shard
    muT = nc.dram_tensor("muT", [IN_F, OS], F16, kind="ExternalInput").ap()
    rhoT = nc.dram_tensor("rhoT", [IN_F, OS], F16, kind="ExternalInput").ap()
    epsT = nc.dram_tensor("epsT", [IN_F, OS], F16, kind="ExternalInput").ap()
    bmu = nc.dram_tensor("bmu", [1, OS], FP32, kind="ExternalInput").ap()
    brho = nc.dram_tensor("brho", [1, OS], FP32, kind="ExternalInput").ap()
    beps = nc.dram_tensor("beps", [1, OS], FP32, kind="ExternalInput").ap()
    ones = nc.dram_tensor("ones", [1, 128], FP32, kind="ExternalInput").ap()
    out = nc.dram_tensor("out", [NS, OS], FP32, kind="ExternalOutput").ap()

    AF = mybir.ActivationFunctionType

    with tile.TileContext(nc) as tc:
        with (
            tc.tile_pool(name="wt", bufs=1) as wt_pool,
            tc.tile_pool(name="bias", bufs=1) as bias_pool,
            tc.tile_pool(name="stage", bufs=2) as stage_pool,
            tc.tile_pool(name="xt", bufs=2) as xt_pool,
            tc.tile_pool(name="outp", bufs=4) as out_pool,
            tc.tile_pool(name="psum", bufs=1, space="PSUM") as psum_pool,
        ):
            # ---- 8 psum accumulators: tag (q, sub) -> one bank each
            def ps_tile(q, sub, s):
                return psum_pool.tile([128, 512], FP32, tag=f"ps{q}{sub}",
                                      name=f"ps_{s}_{q}_{sub}")

            # ---- bias: row [1, OS] then ones-matmul broadcast to [128, OS]
            # (emitted right after pair 0's loads below, so the first weight
            # chunk is already in flight while ACT does the bias softplus)
            ones_t = bias_pool.tile([1, 128], FP32, tag="ones")
            bmu_r = bias_pool.tile([1, OS], FP32, tag="bmu")
            brho_r = bias_pool.tile([1, OS], FP32, tag="brho")
            beps_r = bias_pool.tile([1, OS], FP32, tag="beps")
            bias_t = bias_pool.tile([128, OS], FP32, tag="bias")

            def emit_bias():
                nc.sync.dma_start(ones_t[:], ones[:])
                nc.sync.dma_start(bmu_r[:], bmu[:])
                nc.sync.dma_start(brho_r[:], brho[:])
                nc.sync.dma_start(beps_r[:], beps[:])
                nc.scalar.activation(brho_r[:], brho_r[:], AF.Exp)
                nc.scalar.activation(brho_r[:], brho_r[:], AF.Ln, bias=1.0)
                nc.vector.tensor_mul(beps_r[:], brho_r[:], beps_r[:])
                nc.vector.tensor_add(bmu_r[:], beps_r[:], bmu_r[:])
                for q in range(2):
                    bps = ps_tile(1, 2 + q, -1)  # borrow q1 banks; done early
                    nc.tensor.matmul(bps[:], ones_t[:],
                                     bmu_r[:, q*512:(q+1)*512],
                                     start=True, stop=True)
                    nc.vector.tensor_copy(bias_t[:, q*512:(q+1)*512], bps[:])

            # ---- x panels: per super-tile, 4 k-quarter panels of 8 chunks
            def xt_panel(s, kq):
                xtt = xt_pool.tile([128, KQ * NB], F16, tag=f"kq{kq}",
                                   name=f"xt_s{s}_k{kq}")
                for j in range(KQ):
                    ib = kq * KQ + j
                    nc.sync.dma_start(
                        xtt[:, j * NB:(j + 1) * NB],
                        xT[ib * 128:(ib + 1) * 128, s * NB:(s + 1) * NB])
                return xtt

            def xt_panels(s):
                return [xt_panel(s, kq) for kq in range(NKQ)]

            def xs_slice(panels, ib, sub):
                kq, j = divmod(ib, KQ)
                return panels[kq][:, j * NB + sub * 128:
                                  j * NB + (sub + 1) * 128]

            # ---- weight quads: wts[g][:, (ib%4)*1024 + o] for ib in quad g
            wts = [wt_pool.tile([128, QUAD * OS], F16, tag=f"wt{g}",
                                name=f"wt{g}") for g in range(NQUADS)]

            def w_slice(ib, q):
                g, jj = divmod(ib, QUAD)
                return wts[g][:, jj * OS + q * 512: jj * OS + (q + 1) * 512]

            def prep_pair(p):
                # pair p covers k-blocks 2p, 2p+1 -> half of quad p//2
                g, h = divmod(p, 2)
                rho_s = stage_pool.tile([128, 2 * OS], F16, tag="rho",
                                        name=f"rho{p}")
                mu_s = stage_pool.tile([128, 2 * OS], F16, tag="mu",
                                       name=f"mu{p}")
                eps_s = stage_pool.tile([128, 2 * OS], F16, tag="eps",
                                        name=f"eps{p}")
                for jj in range(2):
                    sl = slice((2*p + jj) * 128, (2*p + jj + 1) * 128)
                    nc.sync.dma_start(rho_s[:, jj*OS:(jj+1)*OS], rhoT[sl, :])
                # mu/eps ride the second HWDGE ring (ACT): doubles dispatch
                # throughput during the startup stream; safe here because the
                # kernel has no transpose DMAs and no SBUF<->SBUF DMAs.
                for jj in range(2):
                    sl = slice((2*p + jj) * 128, (2*p + jj + 1) * 128)
                    nc.scalar.dma_start(mu_s[:, jj*OS:(jj+1)*OS], muT[sl, :])
                    nc.scalar.dma_start(eps_s[:, jj*OS:(jj+1)*OS], epsT[sl, :])
                nc.scalar.activation(rho_s[:], rho_s[:], AF.Exp)
                nc.scalar.activation(rho_s[:], rho_s[:], AF.Ln, bias=1.0)
                nc.vector.tensor_mul(eps_s[:], rho_s[:], eps_s[:])
                nc.vector.tensor_add(wts[g][:, h*2*OS:(h+1)*2*OS],
                                     eps_s[:], mu_s[:])

            def drain(ps, s, q, sub, part=None):
                ot = out_pool.tile([128, 512], FP32, tag="ot",
                                   name=f"ot_{s}_{q}_{sub}")
                nc.vector.tensor_add(ot[:], ps[:],
                                     bias_t[:, q * 512:(q + 1) * 512])
                if part is not None:
                    nc.vector.tensor_add(ot[:], ot[:], part[:])
                row = s * NB + sub * 128
                nc.sync.dma_start(out[row:row + 128, q*512:(q+1)*512], ot[:])

            # fp16 partial accumulators for the half-K startup supers
            parts = {(s, q, sub): stage_pool.tile(
                        [128, 512], F16, tag=f"pt{s}{q}{sub}", bufs=1,
                        name=f"part_{s}_{q}_{sub}")
                     for s in range(3) for q in range(2)
                     for sub in range(SUBS)}

            def q_close(s, q, sub, ps, klo):
                # partial bookkeeping at the end of a quarter-K chain
                if klo == 0:
                    nc.vector.tensor_copy(parts[(s, q, sub)][:], ps[:])
                elif klo < KB - KQ:
                    nc.vector.tensor_add(parts[(s, q, sub)][:],
                                         parts[(s, q, sub)][:], ps[:])
                else:
                    drain(ps, s, q, sub, part=parts[(s, q, sub)])

            def quarter_pass(s, panels, klo, ib_outer):
                khi = klo + KQ
                pss = {(q, sub): ps_tile(q, sub, s * 100 + klo)
                       for q in range(2) for sub in range(SUBS)}
                if ib_outer:
                    # arrival-paced: consume each k-block across all banks
                    for ib in range(klo, khi):
                        for q in range(2):
                            for sub in range(SUBS):
                                nc.tensor.matmul(
                                    pss[(q, sub)][:],
                                    xs_slice(panels, ib, sub),
                                    w_slice(ib, q),
                                    start=(ib == klo), stop=(ib == khi - 1))
                    for q in range(2):
                        for sub in range(SUBS):
                            q_close(s, q, sub, pss[(q, sub)], klo)
                else:
                    # dense: per-bank chains with immediate staggered closes
                    for q in range(2):
                        for sub in range(SUBS):
                            for ib in range(klo, khi):
                                nc.tensor.matmul(
                                    pss[(q, sub)][:],
                                    xs_slice(panels, ib, sub),
                                    w_slice(ib, q),
                                    start=(ib == klo), stop=(ib == khi - 1))
                            q_close(s, q, sub, pss[(q, sub)], klo)

            # ---- startup stream: weight pairs and the x panels they gate,
            # interleaved so each arriving pair feeds quarter-K chains of
            # three supers (s0 arrival-paced, s1/s2 dense+staggered).
            xtq = {0: [None]*NKQ, 1: [None]*NKQ, 2: [None]*NKQ}
            prep_pair(0)
            emit_bias()
            xtq[0][0] = xt_panel(0, 0)
            prep_pair(1)
            xtq[1][0] = xt_panel(1, 0)
            prep_pair(2)
            xtq[2][0] = xt_panel(2, 0)
            prep_pair(3)
            for s in range(3):
                quarter_pass(s, xtq[s], 0, ib_outer=(s == 0))
            xtq[0][1] = xt_panel(0, 1)
            prep_pair(4)
            xtq[1][1] = xt_panel(1, 1)
            prep_pair(5)
            xtq[2][1] = xt_panel(2, 1)
            prep_pair(6)
            prep_pair(7)
            for s in range(3):
                quarter_pass(s, xtq[s], KQ, ib_outer=(s == 0))
            xtq[0][2] = xt_panel(0, 2)
            prep_pair(8)
            xtq[1][2] = xt_panel(1, 2)
            prep_pair(9)
            xtq[2][2] = xt_panel(2, 2)
            prep_pair(10)
            prep_pair(11)
            for s in range(3):
                quarter_pass(s, xtq[s], 2 * KQ, ib_outer=(s == 0))
            xtq[0][3] = xt_panel(0, 3)
            prep_pair(12)
            xtq[1][3] = xt_panel(1, 3)
            prep_pair(13)
            xtq[2][3] = xt_panel(2, 3)
            prep_pair(14)
            prep_pair(15)
            xtq3 = xt_panels(3)
            for s in range(3):
                quarter_pass(s, xtq[s], 3 * KQ, ib_outer=(s == 0))

            # ---- super-tiles 3..: sub-outer phases, per-bank early drains
            panels = xtq3
            for s in range(3, NSUP):
                nxt = xt_panels(s + 1) if s + 1 < NSUP else None
                for q in range(2):
                    for sub in range(SUBS):
                        ps = ps_tile(q, sub, s)
                        for ib in range(KB):
                            nc.tensor.matmul(
                                ps[:], xs_slice(panels, ib, sub),
                                w_slice(ib, q),
                                start=(ib == 0), stop=(ib == KB - 1))
                        drain(ps, s, q, sub)
                panels = nxt

            # dummy matmul: absorbs the PE tail-DRAIN gating so the last
            # real bank's sem fires immediately (saves ~5us of tail)
            dps = psum_pool.tile([128, 512], FP32, tag="ps00", name="ps_dummy")
            nc.tensor.matmul(dps[:], wts[0][:, 0:128], wts[0][:, 0:512],
                             start=True, stop=True)

    nc.compile()
    return nc


_NC = None


def _get_nc():
    global _NC
    if _NC is None:
        _NC = _build_nc()
    return _NC


def kernel(x, weight_mu, weight_rho, bias_mu, bias_rho, eps_w, eps_b,
           _trace=False, _trace_kwargs=None):
    x = np.asarray(x, dtype=np.float32)
    weight_mu = np.asarray(weight_mu, dtype=np.float32)
    weight_rho = np.asarray(weight_rho, dtype=np.float32)
    bias_mu = np.asarray(bias_mu, dtype=np.float32)
    bias_rho = np.asarray(bias_rho, dtype=np.float32)
    eps_w = np.asarray(eps_w, dtype=np.float32)
    eps_b = np.asarray(eps_b, dtype=np.float32)

    nc = _get_nc()
    xT = np.ascontiguousarray(x.T).astype(np.float16)
    muT = np.ascontiguousarray(weight_mu.T).astype(np.float16)
    rhoT = np.ascontiguousarray(weight_rho.T).astype(np.float16)
    epsT = np.ascontiguousarray(eps_w.T).astype(np.float16)
    ones = np.ones((1, 128), np.float32)

    in_maps = []
    for c in range(N_CORES):
        r, q = divmod(c, C)
        osl = slice(q * OS, (q + 1) * OS)
        in_maps.append({
            "xT": np.ascontiguousarray(xT[:, r * NS:(r + 1) * NS]),
            "muT": np.ascontiguousarray(muT[:, osl]),
            "rhoT": np.ascontiguousarray(rhoT[:, osl]),
            "epsT": np.ascontiguousarray(epsT[:, osl]),
            "bmu": bias_mu[osl].reshape(1, OS),
            "brho": bias_rho[osl].reshape(1, OS),
            "beps": eps_b[osl].reshape(1, OS),
            "ones": ones,
        })

    kwargs = {}
    if _trace:
        kwargs["trace"] = True
        if _trace_kwargs:
            kwargs.update(_trace_kwargs)
    res = bass_utils.run_bass_kernel_spmd(
        nc, in_maps, core_ids=list(range(N_CORES)), **kwargs)

    out = np.empty((N, OUT_F), np.float32)
    for c in range(N_CORES):
        r, q = divmod(c, C)
        out[r * NS:(r + 1) * NS, q * OS:(q + 1) * OS] = res.results[c]["out"]
    if _trace:
        return out, res
    return out


# revision 20
# speedup vs baseline: 1.0291x; 1.0291x over previous
"""Bayesian linear layer on 8 TRN2 NeuronCores.

Computes  out = x @ (mu + softplus(rho) * eps_w).T + (bmu + softplus(brho) * eps_b)
for x [16384, 4096], weights [4096, 4096].

Sharding: 2-way split of the batch dim (N) x 4-way split of out_features.
Each core computes an [8192, 1024] fp32 output shard.

Design notes (v2):
  - Weight inputs are shipped host-transposed ([in_f, out_f] fp16), so the
    device materializes W^T = mu + softplus(rho)*eps with cheap LINEAR loads
    (0.6us SP dispatch per chunk vs 1.3us for a DMA transpose) and the
    elementwise softplus/FMA run directly in [i, o] layout.  Weights live in
    8 resident quad tiles [128, 4x1024] fp16; softplus is Exp then Ln(x+1)
    on ACT at FD=4096 to amortize the 352-cycle instruction overhead.
  - x is shipped host-transposed ([in_f, n] fp16) so k-major panels load
    with linear [128 x 1KB] DIRECT2D chunks (~0.6us SP dispatch each) into
    double-buffered k-quarter panels (NB=512 row super-tiles).  A DMA
    transpose dispatch costs a fixed ~1.3us of SP time regardless of rows,
    so 512 of them (measured 656us) would starve the SP ring.
  - Matmuls are fp16, N=512 moving, fp32 PSUM.  Phase = (super-tile, q-half);
    q=0 phases use PSUM banks 0-3, q=1 banks 4-7, and each bank is drained
    (DVE bias-add) right after its 32-matmul k-chain, so phase transitions
    never wait on banks.  Super-tile 0 instead interleaves both q halves
    across all 8 banks in k-arrival order, so the PE starts consuming weight
    quads ~10us in, overlapping the whole prep stream.
  - bias = bmu + softplus(brho)*eps_b is computed on one partition from
    [1, OS] rows and broadcast to [128, OS] with a K=1 ones-matmul.
All DMAs stay on the SP HWDGE ring: splitting across the SP+ACT rings
corrupts results on this stack (completion tracking assumes one ring).
"""

import numpy as np

import bass_rust as _bass_rust
import concourse.bacc as bacc
import concourse.tile as tile
from concourse import mybir
from concourse import bass_utils
from concourse.hw_specs import get_activation_tables


class _Bacc(bacc.Bacc):
    """Bacc whose activation-table placement resolves Exp and Ln to the one
    table set containing both (natural_log_exp_and_others), instead of
    thrashing between per-function sets (one 1.3us ACT_TABLE_LOAD per
    ACTIVATE)."""

    def insert_act_table_loads(self):
        tables = list(get_activation_tables(self.m.arch).items())
        AF = mybir.ActivationFunctionType
        filtered = []
        for name, funcs in tables:
            if name != "natural_log_exp_and_others":
                funcs = funcs - {AF.Exp, AF.Ln}
            filtered.append((name, funcs))
        _bass_rust.insert_act_table_loads(self, filtered)


R, C = 2, 4                      # grid: R-way split of N, C-way split of out_f
N, IN_F, OUT_F = 16384, 4096, 4096
NS, OS = N // R, OUT_F // C      # per-core shards: 8192 rows, 1024 out cols
KB = IN_F // 128                 # 32 k-blocks
NB = 512                         # rows per super-tile
SUBS = NB // 128                 # 4 row-subtiles per super-tile
NSUP = NS // NB                  # 16 super-tiles
NKQ = 4                          # k-quarters per super-tile panel set
KQ = KB // NKQ                   # 8 k-blocks per quarter
QUAD = 4                         # k-blocks per weight quad tile
NQUADS = KB // QUAD              # 8 weight quad tiles
N_CORES = 8

FP32 = mybir.dt.float32
F16 = mybir.dt.float16


def _build_nc():
    nc = _Bacc("TRN2", target_bir_lowering=False, debug=False)

    # x, host-transposed AND pre-tiled: row block (s*KB + ib)*128 + p holds
    # x[s*NB : (s+1)*NB, ib*128 + p] -- every [128, NB] panel chunk is one
    # fully contiguous 128KB DRAM read (strided reads only hit ~70% of HBM
    # peak, and the startup stream is HBM-paced).
    xT = nc.dram_tensor("xT", [NSUP * KB * 128, NB], F16,
                        kind="ExternalInput").ap()
    # host-transposed weight params: [in_f, out_f] for this core's o-shard
    muT = nc.dram_tensor("muT", [IN_F, OS], F16, kind="ExternalInput").ap()
    rhoT = nc.dram_tensor("rhoT", [IN_F, OS], F16, kind="ExternalInput").ap()
    epsT = nc.dram_tensor("epsT", [IN_F, OS], F16, kind="ExternalInput").ap()
    bmu = nc.dram_tensor("bmu", [1, OS], FP32, kind="ExternalInput").ap()
    brho = nc.dram_tensor("brho", [1, OS], FP32, kind="ExternalInput").ap()
    beps = nc.dram_tensor("beps", [1, OS], FP32, kind="ExternalInput").ap()
    ones = nc.dram_tensor("ones", [1, 128], FP32, kind="ExternalInput").ap()
    out = nc.dram_tensor("out", [NS, OS], FP32, kind="ExternalOutput").ap()

    AF = mybir.ActivationFunctionType

    with tile.TileContext(nc) as tc:
        with (
            tc.tile_pool(name="wt", bufs=1) as wt_pool,
            tc.tile_pool(name="bias", bufs=1) as bias_pool,
            tc.tile_pool(name="stage", bufs=2) as stage_pool,
            tc.tile_pool(name="xt", bufs=2) as xt_pool,
            tc.tile_pool(name="outp", bufs=4) as out_pool,
            tc.tile_pool(name="psum", bufs=1, space="PSUM") as psum_pool,
        ):
            # ---- 8 psum accumulators: tag (q, sub) -> one bank each
            def ps_tile(q, sub, s):
                return psum_pool.tile([128, 512], FP32, tag=f"ps{q}{sub}",
                                      name=f"ps_{s}_{q}_{sub}")

            # ---- bias: row [1, OS] then ones-matmul broadcast to [128, OS]
            # (emitted right after pair 0's loads below, so the first weight
            # chunk is already in flight while ACT does the bias softplus)
            ones_t = bias_pool.tile([1, 128], FP32, tag="ones")
            bmu_r = bias_pool.tile([1, OS], FP32, tag="bmu")
            brho_r = bias_pool.tile([1, OS], FP32, tag="brho")
            beps_r = bias_pool.tile([1, OS], FP32, tag="beps")
            bias_t = bias_pool.tile([128, OS], FP32, tag="bias")

            def emit_bias():
                nc.sync.dma_start(ones_t[:], ones[:])
                nc.sync.dma_start(bmu_r[:], bmu[:])
                nc.sync.dma_start(brho_r[:], brho[:])
                nc.sync.dma_start(beps_r[:], beps[:])
                nc.scalar.activation(brho_r[:], brho_r[:], AF.Exp)
                nc.scalar.activation(brho_r[:], brho_r[:], AF.Ln, bias=1.0)
                nc.vector.tensor_mul(beps_r[:], brho_r[:], beps_r[:])
                nc.vector.tensor_add(bmu_r[:], beps_r[:], bmu_r[:])
                for q in range(2):
                    bps = ps_tile(1, 2 + q, -1)  # borrow q1 banks; done early
                    nc.tensor.matmul(bps[:], ones_t[:],
                                     bmu_r[:, q*512:(q+1)*512],
                                     start=True, stop=True)
                    nc.vector.tensor_copy(bias_t[:, q*512:(q+1)*512], bps[:])

            # ---- x panels: per super-tile, 4 k-quarter panels of 8 chunks
            def xt_panel(s, kq):
                xtt = xt_pool.tile([128, KQ * NB], F16, tag=f"kq{kq}",
                                   name=f"xt_s{s}_k{kq}")
                for j in range(KQ):
                    ib = kq * KQ + j
                    row = (s * KB + ib) * 128
                    nc.sync.dma_start(
                        xtt[:, j * NB:(j + 1) * NB],
                        xT[row:row + 128, :])
                return xtt

            def xt_panels(s):
                return [xt_panel(s, kq) for kq in range(NKQ)]

            def xs_slice(panels, ib, sub):
                kq, j = divmod(ib, KQ)
                return panels[kq][:, j * NB + sub * 128:
                                  j * NB + (sub + 1) * 128]

            # ---- weight quads: wts[g][:, (ib%4)*1024 + o] for ib in quad g
            wts = [wt_pool.tile([128, QUAD * OS], F16, tag=f"wt{g}",
                                name=f"wt{g}") for g in range(NQUADS)]

            def w_slice(ib, q):
                g, jj = divmod(ib, QUAD)
                return wts[g][:, jj * OS + q * 512: jj * OS + (q + 1) * 512]

            def prep_pair(p):
                # pair p covers k-blocks 2p, 2p+1 -> half of quad p//2
                g, h = divmod(p, 2)
                rho_s = stage_pool.tile([128, 2 * OS], F16, tag="rho",
                                        name=f"rho{p}")
                mu_s = stage_pool.tile([128, 2 * OS], F16, tag="mu",
                                       name=f"mu{p}")
                eps_s = stage_pool.tile([128, 2 * OS], F16, tag="eps",
                                        name=f"eps{p}")
                for jj in range(2):
                    sl = slice((2*p + jj) * 128, (2*p + jj + 1) * 128)
                    nc.sync.dma_start(rho_s[:, jj*OS:(jj+1)*OS], rhoT[sl, :])
                for jj in range(2):
                    sl = slice((2*p + jj) * 128, (2*p + jj + 1) * 128)
                    nc.sync.dma_start(mu_s[:, jj*OS:(jj+1)*OS], muT[sl, :])
                    nc.sync.dma_start(eps_s[:, jj*OS:(jj+1)*OS], epsT[sl, :])
                nc.scalar.activation(rho_s[:], rho_s[:], AF.Exp)
                nc.scalar.activation(rho_s[:], rho_s[:], AF.Ln, bias=1.0)
                nc.vector.tensor_mul(eps_s[:], rho_s[:], eps_s[:])
                nc.vector.tensor_add(wts[g][:, h*2*OS:(h+1)*2*OS],
                                     eps_s[:], mu_s[:])

            def drain(ps, s, q, sub, part=None):
                ot = out_pool.tile([128, 512], FP32, tag="ot",
                                   name=f"ot_{s}_{q}_{sub}")
                nc.vector.tensor_add(ot[:], ps[:],
                                     bias_t[:, q * 512:(q + 1) * 512])
                if part is not None:
                    nc.vector.tensor_add(ot[:], ot[:], part[:])
                row = s * NB + sub * 128
                nc.sync.dma_start(out[row:row + 128, q*512:(q+1)*512], ot[:])

            # fp16 partial accumulators for the half-K startup supers
            parts = {(s, q, sub): stage_pool.tile(
                        [128, 512], F16, tag=f"pt{s}{q}{sub}", bufs=1,
                        name=f"part_{s}_{q}_{sub}")
                     for s in range(3) for q in range(2)
                     for sub in range(SUBS)}

            def q_close(s, q, sub, ps, klo):
                # partial bookkeeping at the end of a quarter-K chain
                if klo == 0:
                    nc.vector.tensor_copy(parts[(s, q, sub)][:], ps[:])
                elif klo < KB - KQ:
                    nc.vector.tensor_add(parts[(s, q, sub)][:],
                                         parts[(s, q, sub)][:], ps[:])
                else:
                    drain(ps, s, q, sub, part=parts[(s, q, sub)])

            def quarter_pass(s, panels, klo, ib_outer):
                khi = klo + KQ
                pss = {(q, sub): ps_tile(q, sub, s * 100 + klo)
                       for q in range(2) for sub in range(SUBS)}
                if ib_outer:
                    # arrival-paced: consume each k-block across all banks
                    for ib in range(klo, khi):
                        for q in range(2):
                            for sub in range(SUBS):
                                nc.tensor.matmul(
                                    pss[(q, sub)][:],
                                    xs_slice(panels, ib, sub),
                                    w_slice(ib, q),
                                    start=(ib == klo), stop=(ib == khi - 1))
                    for q in range(2):
                        for sub in range(SUBS):
                            q_close(s, q, sub, pss[(q, sub)], klo)
                else:
                    # dense: per-bank chains with immediate staggered closes
                    for q in range(2):
                        for sub in range(SUBS):
                            for ib in range(klo, khi):
                                nc.tensor.matmul(
                                    pss[(q, sub)][:],
                                    xs_slice(panels, ib, sub),
                                    w_slice(ib, q),
                                    start=(ib == klo), stop=(ib == khi - 1))
                            q_close(s, q, sub, pss[(q, sub)], klo)

            # ---- startup stream: weight pairs and the x panels they gate,
            # interleaved so each arriving pair feeds quarter-K chains of
            # three supers (s0 arrival-paced, s1/s2 dense+staggered).
            xtq = {0: [None]*NKQ, 1: [None]*NKQ, 2: [None]*NKQ}
            prep_pair(0)
            emit_bias()
            xtq[0][0] = xt_panel(0, 0)
            prep_pair(1)
            xtq[1][0] = xt_panel(1, 0)
            prep_pair(2)
            xtq[2][0] = xt_panel(2, 0)
            prep_pair(3)
            for s in range(3):
                quarter_pass(s, xtq[s], 0, ib_outer=(s == 0))
            xtq[0][1] = xt_panel(0, 1)
            prep_pair(4)
            xtq[1][1] = xt_panel(1, 1)
            prep_pair(5)
            xtq[2][1] = xt_panel(2, 1)
            prep_pair(6)
            prep_pair(7)
            for s in range(3):
                quarter_pass(s, xtq[s], KQ, ib_outer=(s == 0))
            xtq[0][2] = xt_panel(0, 2)
            prep_pair(8)
            xtq[1][2] = xt_panel(1, 2)
            prep_pair(9)
            xtq[2][2] = xt_panel(2, 2)
            prep_pair(10)
            prep_pair(11)
            for s in range(3):
                quarter_pass(s, xtq[s], 2 * KQ, ib_outer=(s == 0))
            xtq[0][3] = xt_panel(0, 3)
            prep_pair(12)
            xtq[1][3] = xt_panel(1, 3)
            prep_pair(13)
            xtq[2][3] = xt_panel(2, 3)
            prep_pair(14)
            prep_pair(15)
            xtq3 = xt_panels(3)
            for s in range(3):
                quarter_pass(s, xtq[s], 3 * KQ, ib_outer=(s == 0))

            # ---- super-tiles 3..: sub-outer phases, per-bank early drains
            panels = xtq3
            for s in range(3, NSUP):
                nxt = xt_panels(s + 1) if s + 1 < NSUP else None
                for q in range(2):
                    for sub in range(SUBS):
                        ps = ps_tile(q, sub, s)
                        for ib in range(KB):
                            nc.tensor.matmul(
                                ps[:], xs_slice(panels, ib, sub),
                                w_slice(ib, q),
                                start=(ib == 0), stop=(ib == KB - 1))
                        drain(ps, s, q, sub)
                panels = nxt

            # dummy matmul: absorbs the PE tail-DRAIN gating so the last
            # real bank's sem fires immediately (saves ~5us of tail)
            dps = psum_pool.tile([128, 512], FP32, tag="ps00", name="ps_dummy")
            nc.tensor.matmul(dps[:], wts[0][:, 0:128], wts[0][:, 0:512],
                             start=True, stop=True)

    nc.compile()
    return nc


_NC = None


def _get_nc():
    global _NC
    if _NC is None:
        _NC = _build_nc()
    return _NC


def kernel(x, weight_mu, weight_rho, bias_mu, bias_rho, eps_w, eps_b,
           _trace=False, _trace_kwargs=None):
    x = np.asarray(x, dtype=np.float32)
    weight_mu = np.asarray(weight_mu, dtype=np.float32)
    weight_rho = np.asarray(weight_rho, dtype=np.float32)
    bias_mu = np.asarray(bias_mu, dtype=np.float32)
    bias_rho = np.asarray(bias_rho, dtype=np.float32)
    eps_w = np.asarray(eps_w, dtype=np.float32)
    eps_b = np.asarray(eps_b, dtype=np.float32)

    nc = _get_nc()
    x16 = x.astype(np.float16)
    muT = np.ascontiguousarray(weight_mu.T).astype(np.float16)
    rhoT = np.ascontiguousarray(weight_rho.T).astype(np.float16)
    epsT = np.ascontiguousarray(eps_w.T).astype(np.float16)
    ones = np.ones((1, 128), np.float32)

    in_maps = []
    for c in range(N_CORES):
        r, q = divmod(c, C)
        osl = slice(q * OS, (q + 1) * OS)
        in_maps.append({
            # [s, n, ib, p] -> [s, ib, p, n], flattened to [(s*KB+ib)*128+p, n]
            "xT": np.ascontiguousarray(
                x16[r * NS:(r + 1) * NS]
                .reshape(NSUP, NB, KB, 128)
                .transpose(0, 2, 3, 1)
                .reshape(NSUP * KB * 128, NB)),
            "muT": np.ascontiguousarray(muT[:, osl]),
            "rhoT": np.ascontiguousarray(rhoT[:, osl]),
            "epsT": np.ascontiguousarray(epsT[:, osl]),
            "bmu": bias_mu[osl].reshape(1, OS),
            "brho": bias_rho[osl].reshape(1, OS),
            "beps": eps_b[osl].reshape(1, OS),
            "ones": ones,
        })

    kwargs = {}
    if _trace:
        kwargs["trace"] = True
        if _trace_kwargs:
            kwargs.update(_trace_kwargs)
    res = bass_utils.run_bass_kernel_spmd(
        nc, in_maps, core_ids=list(range(N_CORES)), **kwargs)

    out = np.empty((N, OUT_F), np.float32)
    for c in range(N_CORES):
        r, q = divmod(c, C)
        out[r * NS:(r + 1) * NS, q * OS:(q + 1) * OS] = res.results[c]["out"]
    if _trace:
        return out, res
    return out
